# revision 13
# baseline (speedup 1.0000x reference)
"""Bass/Trainium2 kernel for nn_BiLSTM_9028021256417.

Reference computation: 2-layer "bidirectional" LSTM where the fw and bw
chains are independent (no concat between layers), residual add on the
last layer, final output = (fw + bw) / 2.

Sharding (8 NeuronCores, SPMD — identical program, per-core data):
  cores 0-3: forward direction,  batch shards of 128
  cores 4-7: backward direction, batch shards of 128 (host feeds
             time-reversed x, so the device program is direction-agnostic)

Device layout: all state transposed — h, C: [H=128 partitions, B=128 free],
PSUM gate bank z: [128, 4*B] with gate order (g, f, i, o) along free dim.
Matmul inputs bf16; PSUM fp32; elementwise state fp16 (2x DVE mode).

Cell math (per layer):
  gates  = sigmoid(z + b) over all 4 regions in ONE ACT op; the candidate
           region's weights/bias are host-scaled by 2 so that
           tanh(zg) = 2*sigmoid(2*zg) - 1.
  C      = c/2 is the stored cell state (halved):
             t1 = (g~ - 0.5) * i        (fused scalar_tensor_tensor)
             fC = f * C_prev            (tensor_tensor)
             C  = t1 + fC               (tensor_tensor)
  tanh(c) = tanh(2*C) via the ACT op's free input scale.
  h      = o * tanh(c)   (bf16, feeds the PE)

Schedule: layer 1 runs SKEW steps behind layer 0, so per iteration the
ACT queue is [sig0(n), sig1(n-SKEW), tanh0(n), tanh1(n-SKEW)] and layer-1
ops fill layer-0's dependency-chain gaps. PE queue per iteration:
[U0(n+1) (chain-critical), W1(n), W0(n+2) prefetch, U1(n-SKEW+1)].
"""

import numpy as np
import ml_dtypes

import concourse.bass as bass
import concourse.tile as tile
from concourse import bacc, mybir
from concourse.bass_utils import run_bass_kernel_spmd

AF = mybir.ActivationFunctionType
ALU = mybir.AluOpType
FP32 = mybir.dt.float32
FP16 = mybir.dt.float16
BF16 = mybir.dt.bfloat16
NP_BF16 = ml_dtypes.bfloat16

# Problem sizes (hardcoded per the harness contract).
B_TOT, T, E, H = 512, 200, 128, 128
NCORES = 8
NSHARD = 4          # batch shards per direction
B = B_TOT // NSHARD  # 128 per core
P = 128
NG = 4
SKEW = 3            # layer-1 lag in steps

# Device gate order (g, f, i, o) -> Keras 4H order is (i, f, g, o).
KERAS_IDX = [2, 1, 0, 3]  # g, f, i, o
COL_G = slice(0 * B, 1 * B)
COL_F = slice(1 * B, 2 * B)
COL_I = slice(2 * B, 3 * B)
COL_O = slice(3 * B, 4 * B)


def _build_program(scalar_bias: float | None, t_steps: int = T):
    nc = bacc.Bacc("TRN2", target_bir_lowering=False, debug=False)

    xT = nc.dram_tensor("xT", [t_steps, E, B], BF16, kind="ExternalInput").ap()
    w = nc.dram_tensor("w", [2, NG, P, P], BF16, kind="ExternalInput").ap()
    u = nc.dram_tensor("u", [2, NG, P, P], BF16, kind="ExternalInput").ap()
    bias = nc.dram_tensor("bias", [2, NG, 1, P], FP32, kind="ExternalInput").ap()
    out = nc.dram_tensor("out", [t_steps, H, B], FP16, kind="ExternalOutput").ap()

    sb = 0.0 if scalar_bias is None else float(scalar_bias)

    with tile.TileContext(nc) as tc:
        with (
            tc.tile_pool(name="wpool", bufs=1) as wpool,
            tc.tile_pool(name="xpool", bufs=8) as xpool,
            tc.tile_pool(name="z0pool", bufs=3, space="PSUM") as z0pool,
            tc.tile_pool(name="z1pool", bufs=SKEW + 1, space="PSUM") as z1pool,
            tc.tile_pool(name="ys0pool", bufs=2) as ys0pool,
            tc.tile_pool(name="ys1pool", bufs=2) as ys1pool,
            tc.tile_pool(name="tpool", bufs=6) as tpool,
            tc.tile_pool(name="c0pool", bufs=2) as c0pool,
            tc.tile_pool(name="c1pool", bufs=2) as c1pool,
            tc.tile_pool(name="tcpool", bufs=4) as tcpool,
            tc.tile_pool(name="h0pool", bufs=SKEW + 2) as h0pool,
            tc.tile_pool(name="h1pool", bufs=2) as h1pool,
            tc.tile_pool(name="opool", bufs=4) as opool,
        ):
            w_t: dict = {}
            u_t: dict = {}
            b_t: dict = {}
            for l in range(2):
                for g in range(NG):
                    wt = wpool.tile([P, P], BF16, tag=f"w{l}{g}")
                    nc.sync.dma_start(wt[:], w[l, g])
                    w_t[l, g] = wt
                    ut = wpool.tile([P, P], BF16, tag=f"u{l}{g}")
                    nc.sync.dma_start(ut[:], u[l, g])
                    u_t[l, g] = ut
                    if scalar_bias is None:
                        # bias applied via K=1 rank-1 matmuls; lhsT row holds
                        # the (pre-scaled) per-H bias values
                        bt = wpool.tile([1, P], FP32, tag=f"b{l}{g}")
                        nc.sync.dma_start(bt[:], bias[l, g])
                        b_t[l, g] = bt

            ones_rhs = wpool.tile([1, B], FP32, tag="ones_rhs")
            nc.vector.memset(ones_rhs[:], 1.0)
            if scalar_bias is not None:
                # the candidate region needs bias 2*s while the ACT op
                # applies s uniformly: add the missing s via a K=1 matmul
                fix_lhs = wpool.tile([1, P], FP32, tag="fix_lhs")
                nc.vector.memset(fix_lhs[:], sb)

            x_t: dict = {}

            def emit_x(n):
                xt = xpool.tile([P, B], BF16, tag="xt")
                nc.sync.dma_start(xt[:], xT[n])
                x_t[n] = xt

            def emit_w0(n):
                """x-projection matmuls for step n into a fresh z0 bank."""
                z0 = z0pool.tile([P, NG * B], FP32, tag="z0")
                solo = n == 0  # no U0 matmuls will follow for step 0
                # one accumulation group per PSUM bank: start on the first
                # matmul into the tile, stop on the last (bias matmuls follow
                # the W matmuls; U matmuls close non-solo groups next iter)
                for g in range(NG):
                    nc.tensor.matmul(
                        z0[:, g * B : (g + 1) * B],
                        lhsT=w_t[0, g][:], rhs=x_t[n][:],
                        start=(g == 0), stop=False,
                    )
                _emit_bias(z0, 0, stop=solo)
                del x_t[n]
                return z0

            def _emit_bias(z, l, stop):
                """Bias via K=1 matmuls. scalar path: only the g-region fix;
                general path: all four gates. stop=True closes the group
                (solo steps only — otherwise the U matmuls close it)."""
                if scalar_bias is not None:
                    nc.tensor.matmul(
                        z[:, COL_G], lhsT=fix_lhs[:], rhs=ones_rhs[:],
                        start=False, stop=stop,
                    )
                else:
                    for g in range(NG):
                        nc.tensor.matmul(
                            z[:, g * B : (g + 1) * B],
                            lhsT=b_t[l, g][:], rhs=ones_rhs[:],
                            start=False, stop=(stop and g == NG - 1),
                        )

            def emit_u(l, z, h_prev):
                for g in range(NG):
                    nc.tensor.matmul(
                        z[:, g * B : (g + 1) * B],
                        lhsT=u_t[l, g][:], rhs=h_prev[:],
                        start=False, stop=(g == NG - 1),
                    )

            def emit_w1(n, h0):
                z1 = z1pool.tile([P, NG * B], FP32, tag="z1")
                solo = n == 0
                for g in range(NG):
                    nc.tensor.matmul(
                        z1[:, g * B : (g + 1) * B],
                        lhsT=w_t[1, g][:], rhs=h0[:],
                        start=(g == 0), stop=False,
                    )
                _emit_bias(z1, 1, stop=solo)
                return z1

            def sig(l, z, pool):
                ys = pool.tile([P, NG * B], FP16, tag=f"ys{l}")
                nc.scalar.activation(ys[:], z[:], AF.Sigmoid, bias=sb)
                return ys

            def cell(l, ys, c_prev, cpool):
                """C(n) = (g~ - 0.5)*i + f*C(n-1)   (C is the halved cell)"""
                if c_prev is not None:
                    fc = tpool.tile([P, B], FP16, tag=f"fc{l}")
                    nc.vector.tensor_tensor(
                        fc[:], ys[:, COL_F], c_prev[:], ALU.mult
                    )
                t1 = tpool.tile([P, B], FP16, tag=f"t1{l}")
                nc.vector.scalar_tensor_tensor(
                    t1[:], ys[:, COL_G], -0.5, ys[:, COL_I], ALU.add, ALU.mult
                )
                c_new = cpool.tile([P, B], FP16, tag=f"c{l}")
                if c_prev is None:
                    nc.vector.tensor_copy(c_new[:], t1[:])
                else:
                    nc.vector.tensor_tensor(c_new[:], t1[:], fc[:], ALU.add)
                return c_new

            def emit_tc(l, c_new):
                tch = tcpool.tile([P, B], FP16, tag=f"tc{l}")
                nc.scalar.activation(tch[:], c_new[:], AF.Tanh, scale=2.0)
                return tch

            def emit_h(l, ys, tch, pool):
                h_new = pool.tile([P, B], BF16, tag=f"h{l}")
                nc.vector.tensor_tensor(h_new[:], ys[:, COL_O], tch[:], ALU.mult)
                return h_new

            # ---- prologue: x DMAs and x-projections for steps 0 and 1
            emit_x(0)
            emit_x(1)
            z0 = {0: emit_w0(0)}
            if t_steps > 1:
                emit_x(2)
                z0[1] = emit_w0(1)

            c0 = None
            c1 = None
            z1: dict = {}
            h0_hist: dict = {}
            h1_prev = None
            ys1 = None

            for n in range(t_steps + SKEW):
                m = n - SKEW  # layer-1 step processed this iteration

                if n < t_steps:
                    # -- layer 0, step n: ACT sig -> DVE cell
                    ys0 = sig(0, z0.pop(n), ys0pool)
                    c0 = cell(0, ys0, c0, c0pool)

                if 0 <= m < t_steps:
                    # -- layer 1, step m: ACT sig -> DVE cell (fills gaps)
                    ys1 = sig(1, z1.pop(m), ys1pool)
                    c1 = cell(1, ys1, c1, c1pool)

                if n < t_steps:
                    tc0 = emit_tc(0, c0)
                    h0 = emit_h(0, ys0, tc0, h0pool)
                    h0_hist[n] = h0
                    # chain-critical: recurrent matmuls for step n+1 first
                    if n + 1 < t_steps:
                        emit_u(0, z0[n + 1], h0)
                    # layer-1 input projection for step n
                    z1[n] = emit_w1(n, h0)

                if 0 <= m < t_steps:
                    tc1 = emit_tc(1, c1)
                    h1 = emit_h(1, ys1, tc1, h1pool)
                    # residual + inter-layer sum: out = h1 + h0 (fp16)
                    ot = opool.tile([P, B], FP16, tag="ot")
                    nc.vector.tensor_tensor(
                        ot[:], h1[:], h0_hist.pop(m)[:], ALU.add
                    )
                    nc.sync.dma_start(out[m], ot[:])
                    h1_prev = h1

                if n < t_steps:
                    # dep-free prefetch: x DMA + x-projection for step n+2
                    if n + 3 < t_steps:
                        emit_x(n + 3)
                    if n + 2 < t_steps:
                        z0[n + 2] = emit_w0(n + 2)

                if 0 <= m < t_steps and m + 1 < t_steps:
                    # recurrent matmuls for layer 1, step m+1
                    emit_u(1, z1[m + 1], h1_prev)

    nc.compile()
    return nc


_PROGRAM_CACHE: dict = {}


def _get_program(scalar_bias, t_steps: int = T):
    key = (scalar_bias, t_steps)
    if key not in _PROGRAM_CACHE:
        _PROGRAM_CACHE[key] = _build_program(scalar_bias, t_steps)
    return _PROGRAM_CACHE[key]


def _prep_inputs(x, W, U, b, scalar_bias):
    """Build the 8 per-core input maps."""
    in_maps = []
    per_dir = {}
    for d in range(2):
        wd = np.empty((2, NG, P, P), dtype=NP_BF16)
        ud = np.empty((2, NG, P, P), dtype=NP_BF16)
        bd = np.empty((2, NG, 1, P), dtype=np.float32)
        for l in range(2):
            for g in range(NG):
                ks = KERAS_IDX[g]
                # candidate gate computed as tanh(zg) = 2*sigmoid(2*zg) - 1:
                # scale its weights/bias by 2
                sc = 2.0 if g == 0 else 1.0
                wd[l, g] = (sc * W[l, d][:, ks * H : (ks + 1) * H]).astype(NP_BF16)
                ud[l, g] = (sc * U[l, d][:, ks * H : (ks + 1) * H]).astype(NP_BF16)
                bd[l, g, 0, :] = (sc * b[l, d][ks * H : (ks + 1) * H]).astype(
                    np.float32
                )
        per_dir[d] = (wd, ud, bd)

    for core in range(NCORES):
        d = core // NSHARD
        s = core % NSHARD
        xs = x[s * B : (s + 1) * B]           # [B, T, E]
        if d == 1:
            xs = xs[:, ::-1, :]               # time-reverse for backward dir
        xTc = np.ascontiguousarray(np.transpose(xs, (1, 2, 0))).astype(NP_BF16)
        wd, ud, bd = per_dir[d]
        in_maps.append({"xT": xTc, "w": wd, "u": ud, "bias": bd})
    return in_maps


def _postprocess(results, dtype):
    full = np.empty((B_TOT, T, H), dtype=np.float32)
    for s in range(NSHARD):
        fw = np.asarray(results[s]["out"]).astype(np.float32)            # [T, H, B]
        bw = np.asarray(results[NSHARD + s]["out"]).astype(np.float32)
        fw_b = np.transpose(fw, (2, 0, 1))            # [B, T, H]
        bw_b = np.transpose(bw, (2, 0, 1))[:, ::-1, :]
        full[s * B : (s + 1) * B] = (fw_b + bw_b) * 0.5
    return full.astype(dtype)


def run(x, W, U, b, **spmd_kwargs):
    """Run the kernel; returns (output, BassKernelResults)."""
    x = np.asarray(x)
    W = np.asarray(W)
    U = np.asarray(U)
    b = np.asarray(b)
    b0 = float(np.asarray(b).flat[0])
    scalar_bias = b0 if np.all(b == b0) else None
    nc = _get_program(scalar_bias)
    in_maps = _prep_inputs(x, W, U, b, scalar_bias)
    res = run_bass_kernel_spmd(nc, in_maps, core_ids=list(range(NCORES)), **spmd_kwargs)
    out = _postprocess(res.results, x.dtype)
    return out, res


def kernel(x, W, U, b):
    out, _ = run(x, W, U, b)
    return out


# revision 20
# speedup vs baseline: 1.0302x; 1.0302x over previous
"""Bass/Trainium2 kernel for nn_BiLSTM_9028021256417.

Reference computation: 2-layer "bidirectional" LSTM where the fw and bw
chains are independent (no concat between layers), residual add on the
last layer, final output = (fw + bw) / 2.

Sharding (8 NeuronCores, SPMD — identical program, per-core data):
  cores 0-3: forward direction,  batch shards of 128
  cores 4-7: backward direction, batch shards of 128 (host feeds
             time-reversed x, so the device program is direction-agnostic)

Device layout: all state transposed — h, C: [H=128 partitions, B=128 free],
PSUM gate bank z: [128, 4*B] with gate order (g, f, i, o) along free dim.
Matmul inputs bf16; PSUM fp32; elementwise state fp16 (2x DVE mode).

Cell math (per layer):
  gates  = sigmoid(z + b) over all 4 regions in ONE ACT op; the candidate
           region's weights/bias are host-scaled by 2 so that
           tanh(zg) = 2*sigmoid(2*zg) - 1.
  C      = c/2 is the stored cell state (halved):
             t1 = (g~ - 0.5) * i        (fused scalar_tensor_tensor)
             fC = f * C_prev            (tensor_tensor)
             C  = t1 + fC               (tensor_tensor)
  tanh(c) = tanh(2*C) via the ACT op's free input scale.
  h      = o * tanh(c)   (bf16, feeds the PE)

Schedule: layer 1 runs SKEW steps behind layer 0, so per iteration the
ACT queue is [sig0(n), sig1(n-SKEW), tanh0(n), tanh1(n-SKEW)] and layer-1
ops fill layer-0's dependency-chain gaps. PE queue per iteration:
[U0(n+1) (chain-critical), W1(n), W0(n+2) prefetch, U1(n-SKEW+1)].
"""

import numpy as np
import ml_dtypes

import concourse.bass as bass
import concourse.tile as tile
from concourse import bacc, mybir
from concourse.bass_utils import run_bass_kernel_spmd

AF = mybir.ActivationFunctionType
ALU = mybir.AluOpType
FP32 = mybir.dt.float32
FP16 = mybir.dt.float16
BF16 = mybir.dt.bfloat16
NP_BF16 = ml_dtypes.bfloat16

# Problem sizes (hardcoded per the harness contract).
B_TOT, T, E, H = 512, 200, 128, 128
NCORES = 8
NSHARD = 4          # batch shards per direction
B = B_TOT // NSHARD  # 128 per core
P = 128
NG = 4
SKEW = 1            # layer-1 lag in steps (tail deferred one more iteration)

# Device gate order (g, f, i, o) -> Keras 4H order is (i, f, g, o).
KERAS_IDX = [2, 1, 0, 3]  # g, f, i, o
COL_G = slice(0 * B, 1 * B)
COL_F = slice(1 * B, 2 * B)
COL_I = slice(2 * B, 3 * B)
COL_O = slice(3 * B, 4 * B)


def _build_program(scalar_bias: float | None, t_steps: int = T):
    nc = bacc.Bacc("TRN2", target_bir_lowering=False, debug=False)

    xT = nc.dram_tensor("xT", [t_steps, E, B], BF16, kind="ExternalInput").ap()
    w = nc.dram_tensor("w", [2, NG, P, P], BF16, kind="ExternalInput").ap()
    u = nc.dram_tensor("u", [2, NG, P, P], BF16, kind="ExternalInput").ap()
    bias = nc.dram_tensor("bias", [2, NG, 1, P], BF16, kind="ExternalInput").ap()
    out = nc.dram_tensor("out", [t_steps, H, B], FP16, kind="ExternalOutput").ap()

    sb = 0.0 if scalar_bias is None else float(scalar_bias)

    with tile.TileContext(nc) as tc:
        with (
            tc.tile_pool(name="wpool", bufs=1) as wpool,
            tc.tile_pool(name="xpool", bufs=6) as xpool,
            tc.tile_pool(name="z0pool", bufs=3, space="PSUM") as z0pool,
            tc.tile_pool(name="z1pool", bufs=3, space="PSUM") as z1pool,
            tc.tile_pool(name="ys0pool", bufs=3) as ys0pool,
            tc.tile_pool(name="ys1pool", bufs=3) as ys1pool,
            tc.tile_pool(name="tpool", bufs=4) as tpool,
            tc.tile_pool(name="c0pool", bufs=2) as c0pool,
            tc.tile_pool(name="c1pool", bufs=3) as c1pool,
            tc.tile_pool(name="tcpool", bufs=4) as tcpool,
            tc.tile_pool(name="h0pool", bufs=4) as h0pool,
            tc.tile_pool(name="h1pool", bufs=3) as h1pool,
            tc.tile_pool(name="opool", bufs=4) as opool,
        ):
            w_t: dict = {}
            u_t: dict = {}
            b_t: dict = {}
            for l in range(2):
                for g in range(NG):
                    wt = wpool.tile([P, P], BF16, tag=f"w{l}{g}")
                    nc.sync.dma_start(wt[:], w[l, g])
                    w_t[l, g] = wt
                    ut = wpool.tile([P, P], BF16, tag=f"u{l}{g}")
                    nc.sync.dma_start(ut[:], u[l, g])
                    u_t[l, g] = ut
                    if scalar_bias is None:
                        # bias applied via K=1 rank-1 matmuls; lhsT row holds
                        # the (pre-scaled) per-H bias values
                        bt = wpool.tile([1, P], BF16, tag=f"b{l}{g}")
                        nc.sync.dma_start(bt[:], bias[l, g])
                        b_t[l, g] = bt

            ones_rhs = wpool.tile([1, B], BF16, tag="ones_rhs")
            nc.vector.memset(ones_rhs[:], 1.0)
            if scalar_bias is not None:
                # the candidate region needs bias 2*s while the ACT op
                # applies s uniformly: add the missing s via a K=1 matmul
                fix_lhs = wpool.tile([1, P], BF16, tag="fix_lhs")
                nc.vector.memset(fix_lhs[:], sb)

            x_t: dict = {}

            def emit_x(n):
                xt = xpool.tile([P, B], BF16, tag="xt")
                nc.sync.dma_start(xt[:], xT[n])
                x_t[n] = xt

            def emit_w0(n):
                """x-projection matmuls for step n into a fresh z0 bank."""
                z0 = z0pool.tile([P, NG * B], FP32, tag="z0")
                solo = n == 0  # no U0 matmuls will follow for step 0
                # one accumulation group per PSUM bank: start on the first
                # matmul into the tile, stop on the last (bias matmuls follow
                # the W matmuls; U matmuls close non-solo groups next iter)
                for g in range(NG):
                    nc.tensor.matmul(
                        z0[:, g * B : (g + 1) * B],
                        lhsT=w_t[0, g][:], rhs=x_t[n][:],
                        start=(g == 0), stop=False,
                    )
                _emit_bias(z0, 0, stop=solo)
                del x_t[n]
                return z0

            def _emit_bias(z, l, stop):
                """Bias via K=1 matmuls. scalar path: only the g-region fix;
                general path: all four gates. stop=True closes the group
                (solo steps only — otherwise the U matmuls close it)."""
                if scalar_bias is not None:
                    nc.tensor.matmul(
                        z[:, COL_G], lhsT=fix_lhs[:], rhs=ones_rhs[:],
                        start=False, stop=stop,
                    )
                else:
                    for g in range(NG):
                        nc.tensor.matmul(
                            z[:, g * B : (g + 1) * B],
                            lhsT=b_t[l, g][:], rhs=ones_rhs[:],
                            start=False, stop=(stop and g == NG - 1),
                        )

            def emit_u(l, z, h_prev):
                for g in range(NG):
                    nc.tensor.matmul(
                        z[:, g * B : (g + 1) * B],
                        lhsT=u_t[l, g][:], rhs=h_prev[:],
                        start=False, stop=(g == NG - 1),
                    )

            def emit_w1(n, h0):
                z1 = z1pool.tile([P, NG * B], FP32, tag="z1")
                solo = n == 0
                for g in range(NG):
                    nc.tensor.matmul(
                        z1[:, g * B : (g + 1) * B],
                        lhsT=w_t[1, g][:], rhs=h0[:],
                        start=(g == 0), stop=False,
                    )
                _emit_bias(z1, 1, stop=solo)
                return z1

            def sig(l, z, pool):
                ys = pool.tile([P, NG * B], FP16, tag=f"ys{l}")
                nc.scalar.activation(ys[:], z[:], AF.Sigmoid, bias=sb)
                return ys

            def emit_tc(l, c_new):
                tch = tcpool.tile([P, B], FP16, tag=f"tc{l}")
                nc.scalar.activation(tch[:], c_new[:], AF.Tanh, scale=2.0)
                return tch

            def emit_h(l, ys, tch, pool):
                h_new = pool.tile([P, B], BF16, tag=f"h{l}")
                nc.vector.tensor_tensor(h_new[:], ys[:, COL_O], tch[:], ALU.mult)
                return h_new

            # ---- prologue: x DMAs and x-projections for steps 0 and 1
            emit_x(0)
            emit_x(1)
            z0 = {0: emit_w0(0)}
            if t_steps > 1:
                emit_x(2)
                z0[1] = emit_w0(1)

            c0_prev = None
            z1: dict = {}
            h0_hist: dict = {}
            ys1_hist: dict = {}
            c1_hist: dict = {}

            # Steady-state iteration n (engine queues are emission order):
            #   ACT: tc1(mt) | sig0(n) | sig1(m) | tc0(n)
            #   DVE: h1(mt) | t1'0 | c0 | t1'1 | h0 | c1(m)
            #   GPS: fc0 | out(mt) | fc1
            #   PE : U1(m) | U0(n+1) | W1(n)+fix | W0(n+2)+fix
            # where m = n - SKEW (layer-1 head) and mt = m - 1 (layer-1
            # tail, deferred so U1(m) lands early and sig1(m) never stalls
            # the ring).
            for n in range(t_steps + SKEW + 1):
                m = n - SKEW       # layer-1 head step
                mt = m - 1         # layer-1 tail step (deferred)

                h1 = None
                tc1 = None
                if 0 <= mt < t_steps:
                    tc1 = emit_tc(1, c1_hist[mt])
                    h1 = emit_h(1, ys1_hist.pop(mt), tc1, h1pool)
                    if mt + 1 < t_steps:
                        emit_u(1, z1[mt + 1], h1)

                if n < t_steps:
                    # -- layer 0 head: sigmoid + cell update
                    ys0 = sig(0, z0.pop(n), ys0pool)
                    if n > 0:
                        fc0 = tpool.tile([P, B], FP16, tag="fc0")
                        nc.gpsimd.tensor_tensor(
                            fc0[:], ys0[:, COL_F], c0_prev[:], ALU.mult
                        )
                    t10 = tpool.tile([P, B], FP16, tag="t10")
                    nc.vector.scalar_tensor_tensor(
                        t10[:], ys0[:, COL_G], -0.5, ys0[:, COL_I],
                        ALU.add, ALU.mult,
                    )
                    c0 = c0pool.tile([P, B], FP16, tag="c0")
                    if n > 0:
                        nc.vector.tensor_tensor(c0[:], t10[:], fc0[:], ALU.add)
                    else:
                        nc.vector.tensor_copy(c0[:], t10[:])
                    c0_prev = c0

                if 0 <= mt < t_steps:
                    # inter-layer + residual sum for step mt (GPS, off-chain)
                    ot = opool.tile([P, B], FP16, tag="ot")
                    nc.gpsimd.tensor_tensor(
                        ot[:], h1[:], h0_hist.pop(mt)[:], ALU.add
                    )
                    nc.sync.dma_start(out[mt], ot[:])

                if 0 <= m < t_steps:
                    # -- layer 1 head: sigmoid + cell products
                    ys1 = sig(1, z1.pop(m), ys1pool)
                    t11 = tpool.tile([P, B], FP16, tag="t11")
                    nc.vector.scalar_tensor_tensor(
                        t11[:], ys1[:, COL_G], -0.5, ys1[:, COL_I],
                        ALU.add, ALU.mult,
                    )
                    if m > 0:
                        fc1 = tpool.tile([P, B], FP16, tag="fc1")
                        nc.gpsimd.tensor_tensor(
                            fc1[:], ys1[:, COL_F], c1_hist[m - 1][:], ALU.mult
                        )
                    ys1_hist[m] = ys1

                if n < t_steps:
                    # -- layer 0 tail: tanh, h, recurrent + layer-1 matmuls
                    tc0 = emit_tc(0, c0_prev)
                    h0 = emit_h(0, ys0, tc0, h0pool)
                    h0_hist[n] = h0
                    if n + 1 < t_steps:
                        emit_u(0, z0[n + 1], h0)
                    z1[n] = emit_w1(n, h0)

                if 0 <= m < t_steps:
                    # c1 late on DVE (consumed by tc1 next iteration)
                    c1 = c1pool.tile([P, B], FP16, tag="c1")
                    if m > 0:
                        nc.vector.tensor_tensor(c1[:], t11[:], fc1[:], ALU.add)
                    else:
                        nc.vector.tensor_copy(c1[:], t11[:])
                    c1_hist[m] = c1

                if n < t_steps:
                    # dep-free prefetch: x DMA + x-projection for step n+2
                    if n + 3 < t_steps:
                        emit_x(n + 3)
                    if n + 2 < t_steps:
                        z0[n + 2] = emit_w0(n + 2)

                if 0 <= mt < t_steps:
                    del c1_hist[mt]  # last reads (tc1, fc1) are in this iter

    nc.compile()
    return nc


_PROGRAM_CACHE: dict = {}


def _get_program(scalar_bias, t_steps: int = T):
    key = (scalar_bias, t_steps)
    if key not in _PROGRAM_CACHE:
        _PROGRAM_CACHE[key] = _build_program(scalar_bias, t_steps)
    return _PROGRAM_CACHE[key]


def _prep_inputs(x, W, U, b, scalar_bias):
    """Build the 8 per-core input maps."""
    in_maps = []
    per_dir = {}
    for d in range(2):
        wd = np.empty((2, NG, P, P), dtype=NP_BF16)
        ud = np.empty((2, NG, P, P), dtype=NP_BF16)
        bd = np.empty((2, NG, 1, P), dtype=np.float32)
        for l in range(2):
            for g in range(NG):
                ks = KERAS_IDX[g]
                # candidate gate computed as tanh(zg) = 2*sigmoid(2*zg) - 1:
                # scale its weights/bias by 2
                sc = 2.0 if g == 0 else 1.0
                wd[l, g] = (sc * W[l, d][:, ks * H : (ks + 1) * H]).astype(NP_BF16)
                ud[l, g] = (sc * U[l, d][:, ks * H : (ks + 1) * H]).astype(NP_BF16)
                bd[l, g, 0, :] = (sc * b[l, d][ks * H : (ks + 1) * H]).astype(
                    np.float32
                )
        per_dir[d] = (wd, ud, bd)

    for core in range(NCORES):
        d = core // NSHARD
        s = core % NSHARD
        xs = x[s * B : (s + 1) * B]           # [B, T, E]
        if d == 1:
            xs = xs[:, ::-1, :]               # time-reverse for backward dir
        xTc = np.ascontiguousarray(np.transpose(xs, (1, 2, 0))).astype(NP_BF16)
        wd, ud, bd = per_dir[d]
        in_maps.append({"xT": xTc, "w": wd, "u": ud, "bias": bd})
    return in_maps


def _postprocess(results, dtype):
    full = np.empty((B_TOT, T, H), dtype=np.float32)
    for s in range(NSHARD):
        fw = np.asarray(results[s]["out"]).astype(np.float32)            # [T, H, B]
        bw = np.asarray(results[NSHARD + s]["out"]).astype(np.float32)
        fw_b = np.transpose(fw, (2, 0, 1))            # [B, T, H]
        bw_b = np.transpose(bw, (2, 0, 1))[:, ::-1, :]
        full[s * B : (s + 1) * B] = (fw_b + bw_b) * 0.5
    return full.astype(dtype)


def run(x, W, U, b, **spmd_kwargs):
    """Run the kernel; returns (output, BassKernelResults)."""
    x = np.asarray(x)
    W = np.asarray(W)
    U = np.asarray(U)
    b = np.asarray(b)
    b0 = float(np.asarray(b).flat[0])
    scalar_bias = b0 if np.all(b == b0) else None
    nc = _get_program(scalar_bias)
    in_maps = _prep_inputs(x, W, U, b, scalar_bias)
    res = run_bass_kernel_spmd(nc, in_maps, core_ids=list(range(NCORES)), **spmd_kwargs)
    out = _postprocess(res.results, x.dtype)
    return out, res


def kernel(x, W, U, b):
    out, _ = run(x, W, U, b)
    return out
